# revision 5
# baseline (speedup 1.0000x reference)
"""Trainium2 Bass kernel: 4096x4096 fp32 image, 9x9 valid cross-correlation + bias.

Strategy (v3: TensorEngine banded matmul)
-----------------------------------------
Row-shard across 8 cores (520 input rows each incl. halo -> 512 computed
output rows, last row discarded on host). Inside each core the conv runs on
the PE array as banded matmuls:

  out[i, c] = sum_dj sum_di kern[di, dj] * X[i + di, c + dj]

For a row-tile of K=M+8 input rows on partitions, the vertical taps for all
M output rows live in one banded stationary matrix per horizontal offset dj:

  B_dj[k, i] = kern[k - i, dj]   (0 <= k - i <= 8)

so  psum[i, c] (+)= sum_k B_dj[k, i] * X[k, c + dj]  — nine matmuls (dj=0..8)
PSUM-accumulate the full 9x9 conv for an M x 512 output tile. float32r
streams 1 column/cycle at N>=256 (4x faster than plain fp32, full precision);
fp32r ISA restrictions require even free sizes everywhere, hence the 512-row
/ 512-col even tiling with zero-padded edges. Per core: 5 row-tiles x 8
col-chunks x 9 matmuls = 360 matmuls of N=512 (~77us PE time), with DMA
in/out and the PSUM->SBUF bias-copy (scalar engine) overlapped by the Tile
framework.
"""

import numpy as np

H, W = 4096, 4096
KH, KW = 9, 9
NCORES = 8
OH, OW = H - KH + 1, W - KW + 1  # 4088, 4088
RPC = OH // NCORES  # 511 output rows per core actually used
RPCC = 512  # output rows computed per core (even tiling; last row discarded)
IN_ROWS = RPCC + KH - 1  # 520 input rows per core
WPAD = 4104  # padded input width: 8*512 + 8
OWC = 4096  # computed output width (cols >= 4088 discarded)
MT = 120  # output rows per full row-tile (K = MT + 8 = 128 input rows)
NT = 512  # output cols per PSUM bank chunk
NCHUNK = OWC // NT  # 8
ROW_TILES = [(r0, min(MT, RPCC - r0)) for r0 in range(0, RPCC, MT)]


def _build_nc(repeat=1):
    import concourse.bacc as bacc
    import concourse.mybir as mybir
    import concourse.tile as tile

    F32 = mybir.dt.float32
    F32R = mybir.dt.float32r
    ACT = mybir.ActivationFunctionType

    nc = bacc.Bacc("TRN2", target_bir_lowering=False, debug=False)
    Xs = nc.dram_tensor("Xs", [IN_ROWS, WPAD], F32R, kind="ExternalInput")
    Bm = nc.dram_tensor("Bm", [128, KW * MT], F32R, kind="ExternalInput")
    Bias = nc.dram_tensor("Bias", [128, 1], F32, kind="ExternalInput")
    O = nc.dram_tensor("O", [RPCC, OWC], F32, kind="ExternalOutput")

    with tile.TileContext(nc) as tc:
        with (
            tc.tile_pool(name="const", bufs=1) as cpool,
            tc.tile_pool(name="xp", bufs=3) as xpool,
            tc.tile_pool(name="op", bufs=2) as opool,
            tc.tile_pool(name="ps", bufs=1, space="PSUM") as ppool,
        ):
            bm = cpool.tile([128, KW * MT], F32R)
            nc.sync.dma_start(bm[:], Bm[:])
            bias_t = cpool.tile([128, 1], F32)
            nc.sync.dma_start(bias_t[:], Bias[:])
            psum = [
                ppool.tile([128, NT], F32, name=f"ps{cc}", tag=f"ps{cc}")
                for cc in range(NCHUNK)
            ]

            for _ in range(repeat):
                for r0, M in ROW_TILES:
                    K = M + KH - 1
                    xt = xpool.tile([128, WPAD], F32R, tag="x")
                    nc.sync.dma_start(xt[0:K, :], Xs[r0 : r0 + K, :])
                    ot = opool.tile([128, OWC], F32, tag="o")
                    for dj in range(KW):
                        lhsT = bm[0:K, dj * MT : dj * MT + M]
                        for cc in range(NCHUNK):
                            c0 = cc * NT + dj
                            nc.tensor.matmul(
                                psum[cc][0:M, :],
                                lhsT,
                                xt[0:K, c0 : c0 + NT],
                                start=(dj == 0),
                                stop=(dj == KW - 1),
                            )
                    for cc in range(NCHUNK):
                        nc.scalar.activation(
                            ot[0:M, cc * NT : (cc + 1) * NT],
                            psum[cc][0:M, :],
                            ACT.Identity,
                            bias=bias_t[0:M, :],
                        )
                    nc.sync.dma_start(O[r0 : r0 + M, :], ot[0:M, :])

    nc.compile()
    return nc


def _host_inputs(X, kernel, bias):
    """Per-core input maps: padded row stripe + banded stationaries + bias."""
    X = np.asarray(X, dtype=np.float32)
    kern = np.asarray(kernel, dtype=np.float32)
    bias = np.asarray(bias, dtype=np.float32)

    Xpad = np.zeros((NCORES * RPC + IN_ROWS - RPC, WPAD), np.float32)  # 4097 rows
    Xpad[:H, :W] = X

    Bm = np.zeros((128, KW * MT), np.float32)
    idx = np.arange(MT)
    for dj in range(KW):
        for di in range(KH):
            Bm[idx + di, dj * MT + idx] = kern[di, dj]
    Bias = np.full((128, 1), bias[0], np.float32)

    in_maps = []
    for c in range(NCORES):
        stripe = Xpad[c * RPC : c * RPC + IN_ROWS]  # contiguous row view
        in_maps.append({"Xs": stripe, "Bm": Bm, "Bias": Bias})
    return in_maps


_NC_CACHE = {}


def _get_nc(repeat=1):
    if repeat not in _NC_CACHE:
        _NC_CACHE[repeat] = _build_nc(repeat)
    return _NC_CACHE[repeat]


def kernel(X, kernel, bias):
    from concourse.bass_utils import run_bass_kernel_spmd

    nc = _get_nc()
    in_maps = _host_inputs(X, kernel, bias)
    res = run_bass_kernel_spmd(nc, in_maps, core_ids=list(range(NCORES)))
    out = np.empty((OH, OW), np.float32)
    for c in range(NCORES):
        out[c * RPC : (c + 1) * RPC, :] = res.results[c]["O"][:RPC, :OW]
    return out
